# revision 18
# baseline (speedup 1.0000x reference)
"""DINN forward kernel for Trainium2 (Bass/Tile), batch-sharded across 8 NeuronCores.

Reference computation (B=16384, D=512):
    gates  = sigmoid(x @ W.T + b)                       # [B, D]
    linear = sum(gates * x, axis=1)                     # [B]
    quad   = sum_{i<j} iw_ij * x_i * x_j                # [B]
    out    = sigmoid(linear + quad)[:, None]            # [B, 1]

Data-parallel: x split along batch over 8 cores; W, b, U replicated.
No collectives in the forward pass.

Per-core design (batch shard 2048 rows, feature-major orientation: the PE
outputs live as [dout partitions, batch free]):

  gates: fp8 e4m3 DoubleRow matmuls (2 contraction rows/partition, 0.5 cy
         per output row) -> G^T chunk in PSUM.  W is host-packed *16, x *4;
         the sigmoid's scale=1/64 undoes it.
  sigmoid: ACT runs IN-PLACE on the PSUM bank (scale+bias fused).
  quad:  fp16 matmuls accumulate U^T x ON TOP of the sigmoid values already
         in the bank (start=False) -- the (sig + T) add costs zero DVE work.
         U strictly upper-triangular => only 10 of 16 (k,m) blocks run.
  P = (sig+T) * x on DVE (psum x fp16 -> fp16), folds split DVE/Pool,
  partition_all_reduce on Pool, log-odds DMA'd out; final sigmoid on host
  in float64.

Precision (host-validated): fp16 x/U for the quad term and fp8 gates give
rel err ~4e-3 on the final sigmoid output (tolerance 2e-2).
"""
import sys

if "/opt/trn_rl_repo" not in sys.path:
    sys.path.insert(0, "/opt/trn_rl_repo")

import ml_dtypes
import numpy as np

import concourse.tile as tile
from concourse import bacc, bass_isa, mybir
from concourse.bass_utils import run_bass_kernel_spmd

B, D = 16384, 512
NCORES = 8
BC = B // NCORES            # 2048 rows per core
NBT = BC // 512             # 4 batch tiles per core
NK = D // 128               # 4 contraction chunks

f32 = mybir.dt.float32
f16 = mybir.dt.float16
f8 = mybir.dt.float8e4
AF = mybir.ActivationFunctionType
DR = mybir.MatmulPerfMode.DoubleRow

E4M3 = ml_dtypes.float8_e4m3
W_SCALE = 16.0
X_SCALE = 4.0

# strictly-upper-triangular U blocks, in the order the PE consumes them
QUAD_SLOTS = [(k, m) for m in range(NK) for k in range(m + 1)]
SLOT_IDX = {km: s for s, km in enumerate(QUAD_SLOTS)}

_CACHE = {}


def _build():
    nc = bacc.Bacc("TRN2", target_bir_lowering=False, debug=False,
                   num_devices=NCORES)

    d_xh = nc.declare_dram_parameter("xh", [D, BC], f16, isOutput=False)
    d_xq = nc.declare_dram_parameter("xq", [D, BC], f8, isOutput=False)
    # Wq[p, m, h, i, j] = W.T[h*256 + i*128 + p, m*128 + j] * 16
    d_Wq = nc.declare_dram_parameter("Wq", [128, NK, 2, 2, 128], f8,
                                     isOutput=False)
    d_Upk = nc.declare_dram_parameter("Upk", [128, len(QUAD_SLOTS), 128], f16,
                                      isOutput=False)
    d_Id = nc.declare_dram_parameter("Id", [128, 128], f16, isOutput=False)
    d_bias = nc.declare_dram_parameter("bias", [D], f32, isOutput=False)
    d_out = nc.declare_dram_parameter("out", [1, BC], f32, isOutput=True)

    rearr = lambda ap: ap.rearrange("(c p) n -> p c n", p=128)
    xh_r = rearr(d_xh[:, :])
    xq_r = rearr(d_xq[:, :])

    with tile.TileContext(nc) as tc:
        with tc.tile_pool(name="const", bufs=1) as const, \
             tc.tile_pool(name="xin", bufs=4) as xin, \
             tc.tile_pool(name="elt", bufs=2) as elt, \
             tc.tile_pool(name="pt", bufs=1, space="PSUM") as ptp, \
             tc.tile_pool(name="pg", bufs=4, space="PSUM") as pgp:

            Wq_sb = const.tile([128, NK, 2, 2, 128], f8, tag="wq")
            Upk_sb = const.tile([128, len(QUAD_SLOTS), 128], f16, tag="upk")
            Id_sb = const.tile([128, 128], f16, tag="ident")
            bias_sb = const.tile([128, NK], f32, tag="bias")
            ones_sb = const.tile([128, 1], f16, tag="ones")
            dummy_sb = const.tile([1, 8], f32, tag="dummy")
            lo_sb = const.tile([1, 512], f32, tag="lo_sb")

            # preload the sigmoid activation table (1.3us) before it's needed
            nc.vector.memset(dummy_sb, 0.0)
            nc.scalar.activation(dummy_sb[0:1, :], dummy_sb[0:1, :], AF.Sigmoid)
            nc.gpsimd.memset(ones_sb, 1.0)

            # ---- all input DMAs up front, in PE need-order.  xin bufs=4
            # (no buffer reuse) so the in-order DMA queues never block on a
            # write-after-read dependency.  xh goes through the Pool/SWDGE
            # issue path, which runs in parallel with the SP/HWDGE path.
            xqs, xhs = [], []
            for bt in range(NBT):
                xq_t = xin.tile([128, NK, 512], f8, tag="xq", name="xq")
                xh_t = xin.tile([128, NK, 512], f16, tag="xh", name="xh")
                xqs.append(xq_t)
                xhs.append(xh_t)

            nc.sync.dma_start(out=Wq_sb[:, 0:2], in_=d_Wq[:, 0:2])
            nc.sync.dma_start(out=xqs[0][:, 0:2, :], in_=xq_r[:, 0:2, 0:512])
            nc.sync.dma_start(
                out=bias_sb, in_=d_bias[:].rearrange("(c p) -> p c", p=128))
            nc.sync.dma_start(out=Wq_sb[:, 2:NK], in_=d_Wq[:, 2:NK])
            nc.sync.dma_start(out=xqs[0][:, 2:NK, :], in_=xq_r[:, 2:NK, 0:512])
            nc.sync.dma_start(out=xqs[1], in_=xq_r[:, :, 512:1024])
            nc.sync.dma_start(out=Upk_sb, in_=d_Upk[:, :, :])
            nc.sync.dma_start(out=Id_sb, in_=d_Id[:, :])
            nc.sync.dma_start(out=xhs[0][:, 0:2, :], in_=xh_r[:, 0:2, 0:512])
            nc.sync.dma_start(out=xhs[0][:, 2:NK, :], in_=xh_r[:, 2:NK, 0:512])
            nc.sync.dma_start(out=xqs[2], in_=xq_r[:, :, 1024:1536])
            nc.sync.dma_start(out=xhs[1], in_=xh_r[:, :, 512:1024])
            nc.sync.dma_start(out=xqs[3], in_=xq_r[:, :, 1536:2048])
            nc.sync.dma_start(out=xhs[2], in_=xh_r[:, :, 1024:1536])
            nc.sync.dma_start(out=xhs[3], in_=xh_r[:, :, 1536:2048])

            def emit_tile(bt):
                # gates: fp8 DoubleRow into per-chunk pg banks, sigmoid to
                # SBUF fp16.  quads: fp16, accumulate T into pt from zero
                # (start=True) -- fully independent of the gates/sigmoid
                # chain.  The PE then folds sig into the bank via an identity
                # matmul, so no cross-engine ladder ever blocks the PE.
                bsl = slice(bt * 512, (bt + 1) * 512)
                last = bt == NBT - 1
                xq_t, xh_t = xqs[bt], xhs[bt]
                sig = elt.tile([128, NK, 512], f16, tag="sig", name="sig")
                p_all = elt.tile([128, NK, 512], f16, tag="p", name="p_all")
                pt = ptp.tile([128, NK, 512], f32, tag="pt", name="pt")
                for m in range(NK):
                    pg = pgp.tile([128, 512], f32, tag="pg", name="pg")
                    for h in range(2):
                        nc.tensor.matmul(
                            pg, Wq_sb[:, m, h], xq_t[:, 2 * h:2 * h + 2, :],
                            start=(h == 0), stop=(h == 1), perf_mode=DR)
                    nc.scalar.activation(sig[:, m, :], pg, AF.Sigmoid,
                                         bias=bias_sb[:, m:m + 1],
                                         scale=1.0 / (W_SCALE * X_SCALE))
                for m in range(NK):
                    for k in range(m + 1):
                        nc.tensor.matmul(
                            pt[:, m, :], Upk_sb[:, SLOT_IDX[(k, m)], :],
                            xh_t[:, k, :], start=(k == 0), stop=False)
                    nc.tensor.matmul(pt[:, m, :], Id_sb, sig[:, m, :],
                                     start=False, stop=True)
                    nc.vector.tensor_mul(p_all[:, m, :], pt[:, m, :],
                                         xh_t[:, m, :])
                    if last:
                        # partition+chunk reduction on the PE itself (ones
                        # matmuls into the retired chunk-0 bank): only chunk
                        # 3's short chain trails the final matmul
                        nc.tensor.matmul(
                            pt[0:1, 0, :], ones_sb, p_all[:, m, :],
                            start=(m == 0), stop=(m == 3),
                            skip_group_check=True)
                if last:
                    nc.scalar.activation(lo_sb[0:1, :], pt[0:1, 0, :],
                                         AF.Identity)
                    nc.sync.dma_start(out=d_out[0:1, bsl], in_=lo_sb[0:1, :])
                    return
                s4 = elt.tile([128, 2, 512], f16, tag="s4")
                nc.gpsimd.tensor_add(s4, p_all[:, 0:2, :], p_all[:, 2:4, :])
                sred = elt.tile([128, 512], f16, tag="sred", name="sred")
                nc.vector.tensor_add(sred, s4[:, 0, :], s4[:, 1, :])
                par = elt.tile([128, 512], f32, tag="par", name="par")
                nc.gpsimd.partition_all_reduce(par, sred, 128,
                                               bass_isa.ReduceOp.add)
                nc.sync.dma_start(out=d_out[0:1, bsl], in_=par[0:1, :])

            for bt in range(NBT):
                emit_tile(bt)

    nc.compile()
    return nc


def kernel(x, W, b, iw):
    x = np.asarray(x, np.float32)
    W = np.asarray(W, np.float32)
    b = np.asarray(b, np.float32)
    iw = np.asarray(iw, np.float32)

    # host prep: strictly upper-triangular U from iw (row-major i<j order)
    U = np.zeros((D, D), np.float32)
    iu, ju = np.triu_indices(D, k=1)
    U[iu, ju] = iw

    # gates weights: Wq[p, m, h, i, j] = W.T[h*256+i*128+p, m*128+j] * 16
    Wt = np.ascontiguousarray(W.T)
    Wq = (Wt.reshape(2, 2, 128, NK, 128).transpose(2, 3, 0, 1, 4)
          * W_SCALE).astype(E4M3)
    Upk = np.stack([U[k * 128:(k + 1) * 128, m * 128:(m + 1) * 128]
                    for (k, m) in QUAD_SLOTS], axis=1).astype(np.float16)

    xT = x.T                                 # [D, B] view
    shared = {"Wq": np.ascontiguousarray(Wq), "Upk": np.ascontiguousarray(Upk),
              "Id": np.eye(128, dtype=np.float16), "bias": b}
    in_maps = []
    for c in range(NCORES):
        m = dict(shared)
        xs = xT[:, c * BC:(c + 1) * BC]
        m["xh"] = np.ascontiguousarray(xs.astype(np.float16))
        m["xq"] = np.ascontiguousarray((xs * X_SCALE).astype(E4M3))
        in_maps.append(m)

    if "nc" not in _CACHE:
        _CACHE["nc"] = _build()
    nc = _CACHE["nc"]

    res = run_bass_kernel_spmd(nc, in_maps, list(range(NCORES)))
    lo = np.concatenate(
        [res.results[c]["out"][0] for c in range(NCORES)]).astype(np.float64)
    out = 1.0 / (1.0 + np.exp(-np.clip(lo, -708.0, 708.0)))
    return out.reshape(B, 1).astype(np.float32)


# revision 19
# speedup vs baseline: 1.0264x; 1.0264x over previous
"""DINN forward kernel for Trainium2 (Bass/Tile), batch-sharded across 8 NeuronCores.

Reference computation (B=16384, D=512):
    gates  = sigmoid(x @ W.T + b)                       # [B, D]
    linear = sum(gates * x, axis=1)                     # [B]
    quad   = sum_{i<j} iw_ij * x_i * x_j                # [B]
    out    = sigmoid(linear + quad)[:, None]            # [B, 1]

Data-parallel: x split along batch over 8 cores; W, b, U replicated.
No collectives in the forward pass.

Per-core design (batch shard 2048 rows, feature-major orientation: the PE
outputs live as [dout partitions, batch free]):

  gates: fp8 e4m3 DoubleRow matmuls (2 contraction rows/partition, 0.5 cy
         per output row) -> G^T chunk in PSUM.  W is host-packed *16, x *4;
         the sigmoid's scale=1/64 undoes it.
  sigmoid: ACT runs IN-PLACE on the PSUM bank (scale+bias fused).
  quad:  fp16 matmuls accumulate U^T x ON TOP of the sigmoid values already
         in the bank (start=False) -- the (sig + T) add costs zero DVE work.
         U strictly upper-triangular => only 10 of 16 (k,m) blocks run.
  P = (sig+T) * x on DVE (psum x fp16 -> fp16), folds split DVE/Pool,
  partition_all_reduce on Pool, log-odds DMA'd out; final sigmoid on host
  in float64.

Precision (host-validated): fp16 x/U for the quad term and fp8 gates give
rel err ~4e-3 on the final sigmoid output (tolerance 2e-2).
"""
import sys

if "/opt/trn_rl_repo" not in sys.path:
    sys.path.insert(0, "/opt/trn_rl_repo")

import ml_dtypes
import numpy as np

import concourse.tile as tile
from concourse import bacc, bass_isa, mybir
from concourse.bass_utils import run_bass_kernel_spmd

B, D = 16384, 512
NCORES = 8
BC = B // NCORES            # 2048 rows per core
NBT = BC // 512             # 4 batch tiles per core
NK = D // 128               # 4 contraction chunks

f32 = mybir.dt.float32
f16 = mybir.dt.float16
f8 = mybir.dt.float8e4
AF = mybir.ActivationFunctionType
DR = mybir.MatmulPerfMode.DoubleRow

E4M3 = ml_dtypes.float8_e4m3
W_SCALE = 16.0
X_SCALE = 4.0

# strictly-upper-triangular U blocks, in the order the PE consumes them
QUAD_SLOTS = [(k, m) for m in range(NK) for k in range(m + 1)]
SLOT_IDX = {km: s for s, km in enumerate(QUAD_SLOTS)}

_CACHE = {}


def _build():
    nc = bacc.Bacc("TRN2", target_bir_lowering=False, debug=False,
                   num_devices=NCORES)

    d_xh = nc.declare_dram_parameter("xh", [D, BC], f16, isOutput=False)
    d_xq = nc.declare_dram_parameter("xq", [D, BC], f8, isOutput=False)
    # Wq[p, m, h, i, j] = W.T[h*256 + i*128 + p, m*128 + j] * 16
    d_Wq = nc.declare_dram_parameter("Wq", [128, NK, 2, 2, 128], f8,
                                     isOutput=False)
    d_Upk = nc.declare_dram_parameter("Upk", [128, len(QUAD_SLOTS), 128], f16,
                                      isOutput=False)
    d_Id = nc.declare_dram_parameter("Id", [128, 128], f16, isOutput=False)
    d_bias = nc.declare_dram_parameter("bias", [D], f32, isOutput=False)
    d_out = nc.declare_dram_parameter("out", [1, BC], f32, isOutput=True)

    rearr = lambda ap: ap.rearrange("(c p) n -> p c n", p=128)
    xh_r = rearr(d_xh[:, :])
    xq_r = rearr(d_xq[:, :])

    with tile.TileContext(nc) as tc:
        with tc.tile_pool(name="const", bufs=1) as const, \
             tc.tile_pool(name="xin", bufs=4) as xin, \
             tc.tile_pool(name="elt", bufs=2) as elt, \
             tc.tile_pool(name="pt", bufs=1, space="PSUM") as ptp, \
             tc.tile_pool(name="pg", bufs=4, space="PSUM") as pgp:

            Wq_sb = const.tile([128, NK, 2, 2, 128], f8, tag="wq")
            Upk_sb = const.tile([128, len(QUAD_SLOTS), 128], f16, tag="upk")
            Id_sb = const.tile([128, 128], f16, tag="ident")
            bias_sb = const.tile([128, NK], f32, tag="bias")
            ones_sb = const.tile([128, 1], f16, tag="ones")
            dummy_sb = const.tile([1, 8], f32, tag="dummy")
            lo_sb = const.tile([1, 512], f32, tag="lo_sb")

            # preload the sigmoid activation table (1.3us) before it's needed
            nc.vector.memset(dummy_sb, 0.0)
            nc.scalar.activation(dummy_sb[0:1, :], dummy_sb[0:1, :], AF.Sigmoid)
            nc.gpsimd.memset(ones_sb, 1.0)

            # ---- all input DMAs up front, in PE need-order.  xin bufs=4
            # (no buffer reuse) so the in-order DMA queues never block on a
            # write-after-read dependency.  xh goes through the Pool/SWDGE
            # issue path, which runs in parallel with the SP/HWDGE path.
            xqs, xhs = [], []
            for bt in range(NBT):
                xq_t = xin.tile([128, NK, 512], f8, tag="xq", name="xq")
                xh_t = xin.tile([128, NK, 512], f16, tag="xh", name="xh")
                xqs.append(xq_t)
                xhs.append(xh_t)

            nc.sync.dma_start(out=Wq_sb[:, 0:2], in_=d_Wq[:, 0:2])
            nc.sync.dma_start(out=xqs[0][:, 0:2, :], in_=xq_r[:, 0:2, 0:512])
            nc.sync.dma_start(
                out=bias_sb, in_=d_bias[:].rearrange("(c p) -> p c", p=128))
            nc.sync.dma_start(out=Wq_sb[:, 2:NK], in_=d_Wq[:, 2:NK])
            nc.sync.dma_start(out=xqs[0][:, 2:NK, :], in_=xq_r[:, 2:NK, 0:512])
            nc.sync.dma_start(out=xqs[1], in_=xq_r[:, :, 512:1024])
            nc.sync.dma_start(out=Upk_sb, in_=d_Upk[:, :, :])
            nc.sync.dma_start(out=Id_sb, in_=d_Id[:, :])
            nc.sync.dma_start(out=xhs[0][:, 0:2, :], in_=xh_r[:, 0:2, 0:512])
            nc.sync.dma_start(out=xhs[0][:, 2:NK, :], in_=xh_r[:, 2:NK, 0:512])
            nc.sync.dma_start(out=xqs[2], in_=xq_r[:, :, 1024:1536])
            nc.sync.dma_start(out=xhs[1], in_=xh_r[:, :, 512:1024])
            nc.sync.dma_start(out=xqs[3], in_=xq_r[:, :, 1536:2048])
            nc.sync.dma_start(out=xhs[2], in_=xh_r[:, :, 1024:1536])
            nc.sync.dma_start(out=xhs[3], in_=xh_r[:, :, 1536:2048])

            def emit_tile(bt):
                # gates: fp8 DoubleRow into per-chunk pg banks, sigmoid to
                # SBUF fp16.  quads: fp16, accumulate T into pt from zero
                # (start=True) -- fully independent of the gates/sigmoid
                # chain.  The PE then folds sig into the bank via an identity
                # matmul, so no cross-engine ladder ever blocks the PE.
                bsl = slice(bt * 512, (bt + 1) * 512)
                last = bt == NBT - 1
                xq_t, xh_t = xqs[bt], xhs[bt]
                sig = elt.tile([128, NK, 512], f16, tag="sig", name="sig")
                p_all = elt.tile([128, NK, 512], f16, tag="p", name="p_all")
                pt = ptp.tile([128, NK, 512], f32, tag="pt", name="pt")
                for m in range(NK):
                    pg = pgp.tile([128, 512], f32, tag="pg", name="pg")
                    for h in range(2):
                        nc.tensor.matmul(
                            pg, Wq_sb[:, m, h], xq_t[:, 2 * h:2 * h + 2, :],
                            start=(h == 0), stop=(h == 1), perf_mode=DR)
                    nc.scalar.activation(sig[:, m, :], pg, AF.Sigmoid,
                                         bias=bias_sb[:, m:m + 1],
                                         scale=1.0 / (W_SCALE * X_SCALE))
                for m in range(NK):
                    for k in range(m + 1):
                        nc.tensor.matmul(
                            pt[:, m, :], Upk_sb[:, SLOT_IDX[(k, m)], :],
                            xh_t[:, k, :], start=(k == 0), stop=False)
                    nc.tensor.matmul(pt[:, m, :], Id_sb, sig[:, m, :],
                                     start=False, stop=True)
                    nc.vector.tensor_mul(p_all[:, m, :], pt[:, m, :],
                                         xh_t[:, m, :])
                    if last:
                        # partition+chunk reduction on the PE itself (ones
                        # matmuls into the retired chunk-0 bank): only chunk
                        # 3's short chain trails the final matmul
                        nc.tensor.matmul(
                            pt[0:1, 0, :], ones_sb, p_all[:, m, :],
                            start=(m == 0), stop=(m == 3),
                            skip_group_check=True)
                if last:
                    nc.scalar.activation(lo_sb[0:1, :], pt[0:1, 0, :],
                                         AF.Identity)
                    nc.sync.dma_start(out=d_out[0:1, bsl], in_=lo_sb[0:1, :])
                    return
                s4 = elt.tile([128, 2, 512], f16, tag="s4")
                nc.gpsimd.tensor_add(s4, p_all[:, 0:2, :], p_all[:, 2:4, :])
                sred = elt.tile([128, 512], f16, tag="sred", name="sred")
                nc.gpsimd.tensor_add(sred, s4[:, 0, :], s4[:, 1, :])
                par = elt.tile([128, 512], f32, tag="par", name="par")
                nc.gpsimd.partition_all_reduce(par, sred, 128,
                                               bass_isa.ReduceOp.add)
                nc.sync.dma_start(out=d_out[0:1, bsl], in_=par[0:1, :])

            for bt in range(NBT):
                emit_tile(bt)

    nc.compile()
    return nc


def kernel(x, W, b, iw):
    x = np.asarray(x, np.float32)
    W = np.asarray(W, np.float32)
    b = np.asarray(b, np.float32)
    iw = np.asarray(iw, np.float32)

    # host prep: strictly upper-triangular U from iw (row-major i<j order)
    U = np.zeros((D, D), np.float32)
    iu, ju = np.triu_indices(D, k=1)
    U[iu, ju] = iw

    # gates weights: Wq[p, m, h, i, j] = W.T[h*256+i*128+p, m*128+j] * 16
    Wt = np.ascontiguousarray(W.T)
    Wq = (Wt.reshape(2, 2, 128, NK, 128).transpose(2, 3, 0, 1, 4)
          * W_SCALE).astype(E4M3)
    Upk = np.stack([U[k * 128:(k + 1) * 128, m * 128:(m + 1) * 128]
                    for (k, m) in QUAD_SLOTS], axis=1).astype(np.float16)

    xT = x.T                                 # [D, B] view
    shared = {"Wq": np.ascontiguousarray(Wq), "Upk": np.ascontiguousarray(Upk),
              "Id": np.eye(128, dtype=np.float16), "bias": b}
    in_maps = []
    for c in range(NCORES):
        m = dict(shared)
        xs = xT[:, c * BC:(c + 1) * BC]
        m["xh"] = np.ascontiguousarray(xs.astype(np.float16))
        m["xq"] = np.ascontiguousarray((xs * X_SCALE).astype(E4M3))
        in_maps.append(m)

    if "nc" not in _CACHE:
        _CACHE["nc"] = _build()
    nc = _CACHE["nc"]

    res = run_bass_kernel_spmd(nc, in_maps, list(range(NCORES)))
    lo = np.concatenate(
        [res.results[c]["out"][0] for c in range(NCORES)]).astype(np.float64)
    out = 1.0 / (1.0 + np.exp(-np.clip(lo, -708.0, 708.0)))
    return out.reshape(B, 1).astype(np.float32)


# revision 21
# speedup vs baseline: 1.4016x; 1.3656x over previous
"""DINN forward kernel for Trainium2 (Bass/Tile), batch-sharded across 8 NeuronCores.

Reference computation (B=16384, D=512):
    gates  = sigmoid(x @ W.T + b)                       # [B, D]
    linear = sum(gates * x, axis=1)                     # [B]
    quad   = sum_{i<j} iw_ij * x_i * x_j                # [B]
    out    = sigmoid(linear + quad)[:, None]            # [B, 1]

Data-parallel: x split along batch over 8 cores; W, b, U replicated.
No collectives in the forward pass.

Per-core design (batch shard 2048 rows, feature-major orientation: the PE
outputs live as [dout partitions, batch free]):

  gates: fp8 e4m3 DoubleRow matmuls (2 contraction rows/partition, 0.5 cy
         per output row) -> G^T chunk in PSUM.  W is host-packed *16, x *4;
         the sigmoid's scale=1/64 undoes it.
  sigmoid: ACT runs IN-PLACE on the PSUM bank (scale+bias fused).
  quad:  fp16 matmuls accumulate U^T x ON TOP of the sigmoid values already
         in the bank (start=False) -- the (sig + T) add costs zero DVE work.
         U strictly upper-triangular => only 10 of 16 (k,m) blocks run.
  P = (sig+T) * x on DVE (psum x fp16 -> fp16), folds split DVE/Pool,
  partition_all_reduce on Pool, log-odds DMA'd out; final sigmoid on host
  in float64.

Precision (host-validated): fp16 x/U for the quad term and fp8 gates give
rel err ~4e-3 on the final sigmoid output (tolerance 2e-2).
"""
import sys

if "/opt/trn_rl_repo" not in sys.path:
    sys.path.insert(0, "/opt/trn_rl_repo")

import ml_dtypes
import numpy as np

import concourse.tile as tile
from concourse import bacc, bass_isa, mybir
from concourse.bass_utils import run_bass_kernel_spmd

B, D = 16384, 512
NCORES = 8
BC = B // NCORES            # 2048 rows per core
NBT = BC // 512             # 4 batch tiles per core
NK = D // 128               # 4 contraction chunks

f32 = mybir.dt.float32
f16 = mybir.dt.float16
f8 = mybir.dt.float8e4
AF = mybir.ActivationFunctionType
DR = mybir.MatmulPerfMode.DoubleRow

E4M3 = ml_dtypes.float8_e4m3
W_SCALE = 16.0
X_SCALE = 4.0

# strictly-upper-triangular U blocks, in the order the PE consumes them
QUAD_SLOTS = [(k, m) for m in range(NK) for k in range(m + 1)]
SLOT_IDX = {km: s for s, km in enumerate(QUAD_SLOTS)}

_CACHE = {}


def _build():
    nc = bacc.Bacc("TRN2", target_bir_lowering=False, debug=False,
                   num_devices=NCORES)

    d_xh = nc.declare_dram_parameter("xh", [D, BC], f16, isOutput=False)
    d_xq = nc.declare_dram_parameter("xq", [D, BC], f8, isOutput=False)
    # Wq[p, m, h, i, j] = W.T[h*256 + i*128 + p, m*128 + j] * 16
    d_Wq = nc.declare_dram_parameter("Wq", [128, NK, 2, 2, 128], f8,
                                     isOutput=False)
    d_Upk = nc.declare_dram_parameter("Upk", [128, len(QUAD_SLOTS), 128], f16,
                                      isOutput=False)
    d_Id = nc.declare_dram_parameter("Id", [128, 128], f16, isOutput=False)
    d_bias = nc.declare_dram_parameter("bias", [D], f32, isOutput=False)
    d_out = nc.declare_dram_parameter("out", [1, BC], f32, isOutput=True)

    rearr = lambda ap: ap.rearrange("(c p) n -> p c n", p=128)
    xh_r = rearr(d_xh[:, :])
    xq_r = rearr(d_xq[:, :])

    with tile.TileContext(nc) as tc:
        with tc.tile_pool(name="const", bufs=1) as const, \
             tc.tile_pool(name="xin", bufs=4) as xin, \
             tc.tile_pool(name="elt", bufs=2) as elt, \
             tc.tile_pool(name="pt", bufs=4, space="PSUM") as ptp, \
             tc.tile_pool(name="pg", bufs=4, space="PSUM") as pgp:

            Wq_sb = const.tile([128, NK, 2, 2, 128], f8, tag="wq")
            Upk_sb = const.tile([128, len(QUAD_SLOTS), 128], f16, tag="upk")
            Id_sb = const.tile([128, 128], f16, tag="ident")
            bias_sb = const.tile([128, NK], f32, tag="bias")
            ones_sb = const.tile([128, 1], f16, tag="ones")
            dummy_sb = const.tile([1, 8], f32, tag="dummy")
            lo_sb = const.tile([1, 512], f32, tag="lo_sb")

            # preload the sigmoid activation table (1.3us) before it's needed
            nc.vector.memset(dummy_sb, 0.0)
            nc.scalar.activation(dummy_sb[0:1, :], dummy_sb[0:1, :], AF.Sigmoid)
            nc.gpsimd.memset(ones_sb, 1.0)

            # ---- all input DMAs up front, in PE need-order.  xin bufs=4
            # (no buffer reuse) so the in-order DMA queues never block on a
            # write-after-read dependency.  xh goes through the Pool/SWDGE
            # issue path, which runs in parallel with the SP/HWDGE path.
            xqs, xhs = [], []
            for bt in range(NBT):
                xq_t = xin.tile([128, NK, 512], f8, tag="xq", name="xq")
                xh_t = xin.tile([128, NK, 512], f16, tag="xh", name="xh")
                xqs.append(xq_t)
                xhs.append(xh_t)

            nc.sync.dma_start(out=Wq_sb[:, 0:2], in_=d_Wq[:, 0:2])
            nc.sync.dma_start(out=xqs[0][:, 0:2, :], in_=xq_r[:, 0:2, 0:512])
            nc.sync.dma_start(
                out=bias_sb, in_=d_bias[:].rearrange("(c p) -> p c", p=128))
            nc.sync.dma_start(out=Wq_sb[:, 2:NK], in_=d_Wq[:, 2:NK])
            nc.sync.dma_start(out=xqs[0][:, 2:NK, :], in_=xq_r[:, 2:NK, 0:512])
            nc.sync.dma_start(out=xqs[1], in_=xq_r[:, :, 512:1024])
            nc.sync.dma_start(out=Upk_sb, in_=d_Upk[:, :, :])
            nc.sync.dma_start(out=Id_sb, in_=d_Id[:, :])
            nc.sync.dma_start(out=xhs[0][:, 0:2, :], in_=xh_r[:, 0:2, 0:512])
            nc.sync.dma_start(out=xhs[0][:, 2:NK, :], in_=xh_r[:, 2:NK, 0:512])
            nc.sync.dma_start(out=xqs[2], in_=xq_r[:, :, 1024:1536])
            nc.sync.dma_start(out=xhs[1], in_=xh_r[:, :, 512:1024])
            nc.sync.dma_start(out=xqs[3], in_=xq_r[:, :, 1536:2048])
            nc.sync.dma_start(out=xhs[2], in_=xh_r[:, :, 1024:1536])
            nc.sync.dma_start(out=xhs[3], in_=xh_r[:, :, 1536:2048])

            def emit_tile(bt):
                # gates: fp8 DoubleRow into per-chunk pg banks, sigmoid to
                # SBUF fp16.  quads: fp16, accumulate T into pt from zero
                # (start=True) -- fully independent of the gates/sigmoid
                # chain.  The PE then folds sig into the bank via an identity
                # matmul, so no cross-engine ladder ever blocks the PE.
                bsl = slice(bt * 512, (bt + 1) * 512)
                last = bt == NBT - 1
                xq_t, xh_t = xqs[bt], xhs[bt]
                sig = elt.tile([128, NK, 512], f16, tag="sig", name="sig")
                p_all = elt.tile([128, NK, 512], f16, tag="p", name="p_all")
                # one PSUM tile per chunk: keeps start=True writes (and their
                # dependency tracking) bank-granular, so chunk pipelines never
                # serialize against each other
                pts = [ptp.tile([128, 512], f32, tag="pt", name="pt")
                       for _ in range(NK)]
                for m in range(NK):
                    pg = pgp.tile([128, 512], f32, tag="pg", name="pg")
                    for h in range(2):
                        nc.tensor.matmul(
                            pg, Wq_sb[:, m, h], xq_t[:, 2 * h:2 * h + 2, :],
                            start=(h == 0), stop=(h == 1), perf_mode=DR)
                    nc.scalar.activation(sig[:, m, :], pg, AF.Sigmoid,
                                         bias=bias_sb[:, m:m + 1],
                                         scale=1.0 / (W_SCALE * X_SCALE))
                for m in range(NK):
                    for k in range(m + 1):
                        nc.tensor.matmul(
                            pts[m], Upk_sb[:, SLOT_IDX[(k, m)], :],
                            xh_t[:, k, :], start=(k == 0), stop=False)
                    nc.tensor.matmul(pts[m], Id_sb, sig[:, m, :],
                                     start=False, stop=True)
                    nc.vector.tensor_mul(p_all[:, m, :], pts[m],
                                         xh_t[:, m, :])
                    if last:
                        # partition+chunk reduction on the PE itself (ones
                        # matmuls into the retired chunk-0 bank): only chunk
                        # 3's short chain trails the final matmul
                        nc.tensor.matmul(
                            pts[0][0:1, :], ones_sb, p_all[:, m, :],
                            start=(m == 0), stop=(m == 3),
                            skip_group_check=True)
                if last:
                    nc.scalar.activation(lo_sb[0:1, :], pts[0][0:1, :],
                                         AF.Identity)
                    nc.sync.dma_start(out=d_out[0:1, bsl], in_=lo_sb[0:1, :])
                    return
                s4 = elt.tile([128, 2, 512], f16, tag="s4")
                nc.gpsimd.tensor_add(s4, p_all[:, 0:2, :], p_all[:, 2:4, :])
                sred = elt.tile([128, 512], f16, tag="sred", name="sred")
                nc.gpsimd.tensor_add(sred, s4[:, 0, :], s4[:, 1, :])
                par = elt.tile([128, 512], f32, tag="par", name="par")
                nc.gpsimd.partition_all_reduce(par, sred, 128,
                                               bass_isa.ReduceOp.add)
                nc.sync.dma_start(out=d_out[0:1, bsl], in_=par[0:1, :])

            for bt in range(NBT):
                emit_tile(bt)

    nc.compile()
    return nc


def kernel(x, W, b, iw):
    x = np.asarray(x, np.float32)
    W = np.asarray(W, np.float32)
    b = np.asarray(b, np.float32)
    iw = np.asarray(iw, np.float32)

    # host prep: strictly upper-triangular U from iw (row-major i<j order)
    U = np.zeros((D, D), np.float32)
    iu, ju = np.triu_indices(D, k=1)
    U[iu, ju] = iw

    # gates weights: Wq[p, m, h, i, j] = W.T[h*256+i*128+p, m*128+j] * 16
    Wt = np.ascontiguousarray(W.T)
    Wq = (Wt.reshape(2, 2, 128, NK, 128).transpose(2, 3, 0, 1, 4)
          * W_SCALE).astype(E4M3)
    Upk = np.stack([U[k * 128:(k + 1) * 128, m * 128:(m + 1) * 128]
                    for (k, m) in QUAD_SLOTS], axis=1).astype(np.float16)

    xT = x.T                                 # [D, B] view
    shared = {"Wq": np.ascontiguousarray(Wq), "Upk": np.ascontiguousarray(Upk),
              "Id": np.eye(128, dtype=np.float16), "bias": b}
    in_maps = []
    for c in range(NCORES):
        m = dict(shared)
        xs = xT[:, c * BC:(c + 1) * BC]
        m["xh"] = np.ascontiguousarray(xs.astype(np.float16))
        m["xq"] = np.ascontiguousarray((xs * X_SCALE).astype(E4M3))
        in_maps.append(m)

    if "nc" not in _CACHE:
        _CACHE["nc"] = _build()
    nc = _CACHE["nc"]

    res = run_bass_kernel_spmd(nc, in_maps, list(range(NCORES)))
    lo = np.concatenate(
        [res.results[c]["out"][0] for c in range(NCORES)]).astype(np.float64)
    out = 1.0 / (1.0 + np.exp(-np.clip(lo, -708.0, 708.0)))
    return out.reshape(B, 1).astype(np.float32)
